# revision 5
# baseline (speedup 1.0000x reference)
"""BitLinear (ternary-weight + 8-bit-activation quantized matmul) on 8 TRN2 cores.

Strategy: data-parallel over tokens. Each core gets 2048 of the 16384 tokens
plus the full weight matrix, computes the whole BitLinear forward for its
token shard on device, and the host concatenates the shards.

Math (must match the jax reference):
  w_scale = max(mean(|W|), 1e-6)                       (scalar)
  w_q     = clip(round(W / w_scale), -1, 1)            (ternary)
  a       = clip(max_i |x|, 1e-8, inf)                 (per token)
  x_q     = clip(round(x * 127 / a), -127, 127)        (8-bit ints)
  y       = (x_q @ w_q^T) * w_scale * a / 127

All rounding uses the fp32 magic-number trick (v + 1.5*2^23 - 1.5*2^23 is
round-to-nearest-even). x_q (ints <= 127) is exact in bf16; w_q ({-1,0,1})
is exact in fp8e4; their products accumulate exactly in fp32 PSUM, so the
TensorE matmul (bf16 stationary x fp8 moving — verified exact on HW) is
exact. y is stored bf16 (~3e-3 max-normalized rounding, budget is 2e-2).

Schedule (single pass over W, GEMM starts ~50us in):
  pass 1   load all 16 W row-tiles once, |.|-row-sums on DVE. The last 4
           tiles stay resident in SBUF; the rest stream through a small pool.
  scale    partition-reduce (GpSimd) + scalar chain -> w_scale, 1/w_scale.
  pass 2a  quantize the 4 resident tiles (quarter no=3) with zero DMA:
           scalar act (round magic) -> DVE clip -> GpSimd sub -> bf16 DMA
           transpose -> GpSimd fp8 convert. GEMM can start here.
  pass 2b  re-read j8..11, 4..7, 0..3 (quarters 2,1,0) interleaved with the
           x-tile preps; deadlines are one GEMM sweep apart so the re-reads
           ride spare DMA bandwidth behind the x loads.
  main     for no in 3,2,1,0: for t in 0..15: 16 matmuls -> PSUM, DVE
           drain *(a*w_scale/127) -> bf16 y store.
"""

from contextlib import ExitStack

import numpy as np

import concourse.bass as bass
import concourse.tile as tile
from concourse import bacc, bass_isa, mybir
from concourse.bass import ds, ts
from concourse.bass_utils import run_bass_kernel_spmd

F32 = mybir.dt.float32
BF16 = mybir.dt.bfloat16
FP8 = mybir.dt.float8e4
AF = mybir.ActivationFunctionType
OP = mybir.AluOpType
AX = mybir.AxisListType

B, S, D_IN, D_OUT = 4, 4096, 2048, 2048
N_CORES = 8
TOK = B * S                # 16384 tokens
TPC = TOK // N_CORES       # 2048 tokens per core
NT = TPC // 128            # 16 token tiles per core
NJ = D_OUT // 128          # 16 weight row tiles
NI = D_IN // 128           # 16 contraction (k) blocks
NO = D_OUT // 512          # 4 output column blocks
CM = 12582912.0            # 1.5 * 2^23: fp32 RNE rounding magic
QMAX = 127.0

KNOBS = {
    "ld_bufs": 3,          # streamed W tiles
    "ldx_bufs": 2,         # streamed x tiles
    "wq_bufs": 2,          # quantized-W bf16 staging
    "xq_bufs": 2,          # quantized-x bf16 staging
    "wqtb_bufs": 2,        # post-transpose bf16 W staging (pre fp8 convert)
    "ys_bufs": 3,
    "n_res": 4,            # W tiles kept resident from pass 1 (quarter no=3)
    "pref_pass1": 2,       # x preps emitted inside pass 1
    "tpose_ring": "scalar",
}

_CACHE = {}


def _emit(tc: tile.TileContext, x_d: bass.AP, w_d: bass.AP, y_d: bass.AP):
    nc = tc.nc
    K = KNOBS
    with ExitStack() as ctx:
        ld = ctx.enter_context(tc.tile_pool(name="ld", bufs=K["ld_bufs"]))
        ldx = ctx.enter_context(tc.tile_pool(name="ldx", bufs=K["ldx_bufs"]))
        wres = ctx.enter_context(tc.tile_pool(name="wres", bufs=1))
        wqp = ctx.enter_context(tc.tile_pool(name="wqp", bufs=K["wq_bufs"]))
        xqp = ctx.enter_context(tc.tile_pool(name="xqp", bufs=K["xq_bufs"]))
        wqtb = ctx.enter_context(tc.tile_pool(name="wqtb", bufs=K["wqtb_bufs"]))
        wqtp = ctx.enter_context(tc.tile_pool(name="wqtp", bufs=1))
        xqtp = ctx.enter_context(tc.tile_pool(name="xqtp", bufs=1))
        ysp = ctx.enter_context(tc.tile_pool(name="ysp", bufs=K["ys_bufs"]))
        stats = ctx.enter_context(tc.tile_pool(name="stats", bufs=6))
        consts = ctx.enter_context(tc.tile_pool(name="consts", bufs=1))
        wsp = ctx.enter_context(tc.tile_pool(name="wsp", bufs=1))
        psum = ctx.enter_context(
            tc.tile_pool(name="psum", bufs=8, space=bass.MemorySpace.PSUM)
        )

        tpose = nc.scalar if K["tpose_ring"] == "scalar" else nc.sync
        cpos = consts.tile([128, 1], F32, tag="cpos")
        nc.vector.memset(cpos, CM)

        NRES = K["n_res"]
        jres0 = NJ - NRES                       # first resident j (12)

        # persistent per-token-tile tiles
        xqT = [
            xqtp.tile([128, NI, 128], BF16, tag=f"xqT{t}", name=f"xqT{t}")
            for t in range(NT)
        ]
        souts = [
            consts.tile([128, 1], F32, tag=f"sout{t}", name=f"sout{t}")
            for t in range(NT)
        ]
        # fp8 transposed quantized W, one [128, NI, 512] tile per 512-col block
        wqT = [
            wqtp.tile([128, NI, 512], FP8, tag=f"wqT{no}", name=f"wqT{no}")
            for no in range(NO)
        ]

        ws127 = consts.tile([128, 1], F32, tag="ws127")
        rws = consts.tile([128, 1], F32, tag="rws")

        a_tiles = {}

        def x_prep(t, ws_ready=True):
            """load x tile t, compute scales, quantize, transpose to xqT[t]."""
            xt = ldx.tile([128, D_IN], F32, tag="ldx", name=f"xt{t}")
            nc.sync.dma_start(xt, x_d[ts(t, 128), :])
            a = consts.tile([128, 1], F32, tag=f"xa{t}", name=f"xa{t}")
            a_tiles[t] = a
            nc.vector.reduce_max(a, xt, axis=AX.X, apply_absolute_value=True)
            nc.vector.tensor_scalar(a, a, 1e-8, None, OP.max)
            r0 = stats.tile([128, 1], F32, tag="xr0", name=f"xr0{t}")
            nc.vector.reciprocal(r0, a)
            ntt = stats.tile([128, 1], F32, tag="xntt", name=f"xntt{t}")
            nc.vector.tensor_mul(ntt, a, r0)
            nc.vector.tensor_scalar(ntt, ntt, -1.0, 2.0, OP.mult, OP.add)
            s = stats.tile([128, 1], F32, tag="xs", name=f"xs{t}")
            nc.vector.tensor_mul(s, r0, ntt)
            nc.vector.tensor_scalar(s, s, QMAX, None, OP.mult)  # 127/a
            if ws_ready:
                nc.vector.tensor_mul(souts[t], a, ws127)  # a * w_scale / 127

            # t1 = x*(127/a) + CM  (in-place; RNE round via fp32 magic add)
            nc.scalar.activation(xt, xt, AF.Identity, bias=cpos, scale=s)
            xq = xqp.tile([128, D_IN], BF16, tag="xq", name=f"xq{t}")
            nc.vector.tensor_scalar(xq, xt, -CM, None, OP.add)
            tpose.dma_start(xqT[t], xq, transpose=True)

        def w_quant(j, wt):
            """quantize W row-tile j (already in SBUF as wt) into wqT."""
            no, jq = j // 4, j % 4
            # t1 = W/ws + CM (in-place), clip in offset domain, -CM -> bf16
            nc.scalar.activation(wt, wt, AF.Identity, bias=cpos, scale=rws)
            nc.vector.tensor_scalar(wt, wt, CM - 1.0, CM + 1.0, OP.max, OP.min)
            wq = wqp.tile([128, D_IN], BF16, tag="wq", name=f"wq{j}")
            nc.gpsimd.tensor_scalar(wq, wt, -CM, None, OP.add)
            wb = wqtb.tile([128, NI, 128], BF16, tag="wqtb", name=f"wqtb{j}")
            tpose.dma_start(wb, wq, transpose=True)
            nc.gpsimd.tensor_scalar(
                wqT[no][:, :, ds(jq * 128, 128)], wb, 0.0, None, OP.add
            )

        # ---- pass 1: single read of W, |.| row-sums on DVE ----
        wsums = wsp.tile([128, NJ], F32, tag="wsums")
        wres_tiles = {}
        for j in range(NJ):
            if j >= jres0:
                wt = wres.tile([128, D_IN], F32, tag=f"wres{j}", name=f"wres{j}")
                wres_tiles[j] = wt
            else:
                wt = ld.tile([128, D_IN], F32, tag="ld", name=f"wp1_{j}")
            nc.sync.dma_start(wt, w_d[ts(j, 128), :])
            nc.vector.reduce_sum(
                wsums[:, ds(j, 1)], wt, axis=AX.X, apply_absolute_value=True
            )
            if j == 7 and K["pref_pass1"] >= 1:
                x_prep(0, ws_ready=False)
            if j == 11 and K["pref_pass1"] >= 2:
                x_prep(1, ws_ready=False)

        # ---- w_scale = max(mean|W|, 1e-6), rws ~ 1/w_scale (Newton) ----
        wsum_p = stats.tile([128, 1], F32, tag="wsp")
        nc.vector.reduce_sum(wsum_p, wsums, axis=AX.X)
        wsum_all = stats.tile([128, 1], F32, tag="wsa")
        nc.gpsimd.partition_all_reduce(wsum_all, wsum_p, 128, bass_isa.ReduceOp.add)
        wscale = stats.tile([128, 1], F32, tag="wscale")
        nc.vector.tensor_scalar(
            wscale, wsum_all, 1.0 / (D_OUT * D_IN), 1e-6, OP.mult, OP.max
        )
        r0 = stats.tile([128, 1], F32, tag="wr0")
        nc.vector.reciprocal(r0, wscale)
        ntt = stats.tile([128, 1], F32, tag="wntt")
        nc.vector.tensor_mul(ntt, wscale, r0)
        nc.vector.tensor_scalar(ntt, ntt, -1.0, 2.0, OP.mult, OP.add)
        nc.vector.tensor_mul(rws, r0, ntt)
        nc.vector.tensor_scalar(ws127, wscale, 1.0 / QMAX, None, OP.mult)
        for t in range(min(K["pref_pass1"], NT)):
            nc.vector.tensor_mul(souts[t], a_tiles[t], ws127)

        # ---- pass 2a: resident quarter (no=3), zero DMA on critical path ----
        for j in range(jres0, NJ):
            w_quant(j, wres_tiles[j])

        x_prep(2)
        x_prep(3)

        # ---- pass 2b: re-read the rest in sweep order 2, 1, 0;
        #      interleave remaining x preps (x loads get ring priority) ----
        def w_requant(j):
            wt = ld.tile([128, D_IN], F32, tag="ld", name=f"wp2_{j}")
            nc.sync.dma_start(wt, w_d[ts(j, 128), :])
            w_quant(j, wt)

        xi = 4
        for jg in (range(8, 12), range(4, 8), range(0, 4)):
            for j in jg:
                if xi < NT:
                    x_prep(xi)
                    xi += 1
                w_requant(j)
        while xi < NT:
            x_prep(xi)
            xi += 1

        # ---- main GEMM: no-major sweeps, newest quarter first ----
        for no in range(NO - 1, -1, -1):
            for t in range(NT):
                ps = psum.tile([128, 512], F32, tag="ps")
                for b in range(NI):
                    nc.tensor.matmul(
                        ps,
                        xqT[t][:, b, :],
                        wqT[no][:, b, :],
                        start=(b == 0),
                        stop=(b == NI - 1),
                    )
                ys = ysp.tile([128, 512], BF16, tag="ys")
                nc.vector.tensor_scalar(ys, ps, souts[t], None, OP.mult)
                nc.sync.dma_start(y_d[ts(t, 128), ds(no * 512, 512)], ys)


def _build():
    key = tuple(sorted((k, str(v)) for k, v in KNOBS.items()))
    if key in _CACHE:
        return _CACHE[key]
    nc = bacc.Bacc(
        "TRN2", target_bir_lowering=False, debug=False, num_devices=N_CORES
    )
    x_d = nc.dram_tensor("x", [TPC, D_IN], F32, kind="ExternalInput").ap()
    w_d = nc.dram_tensor("w", [D_OUT, D_IN], F32, kind="ExternalInput").ap()
    y_d = nc.dram_tensor("y", [TPC, D_OUT], BF16, kind="ExternalOutput").ap()
    with tile.TileContext(nc) as tc:
        _emit(tc, x_d, w_d, y_d)
    nc.compile()
    _CACHE[key] = nc
    return nc


_last_result = None  # BassKernelResults of the most recent run (for profiling)


def kernel(x: np.ndarray, weight: np.ndarray, trace: bool = False) -> np.ndarray:
    global _last_result
    nc = _build()
    xf = np.ascontiguousarray(x.reshape(TOK, D_IN), dtype=np.float32)
    wf = np.ascontiguousarray(weight, dtype=np.float32)
    in_maps = [
        {"x": xf[c * TPC:(c + 1) * TPC], "w": wf} for c in range(N_CORES)
    ]
    res = run_bass_kernel_spmd(nc, in_maps, list(range(N_CORES)), trace=trace)
    _last_result = res
    y = np.concatenate(
        [np.asarray(res.results[c]["y"]) for c in range(N_CORES)], axis=0
    )
    return y.astype(np.float32).reshape(B, S, D_OUT)


# revision 7
# speedup vs baseline: 2.4100x; 2.4100x over previous
"""BitLinear (ternary-weight + 8-bit-activation quantized matmul) on 8 TRN2 cores.

Strategy: data-parallel over tokens. Each core gets 2048 of the 16384 tokens
plus the full weight matrix, computes the whole BitLinear forward for its
token shard on device, and the host concatenates the shards.

Math (must match the jax reference):
  w_scale = max(mean(|W|), 1e-6)                       (scalar)
  w_q     = clip(round(W / w_scale), -1, 1)            (ternary)
  a       = clip(max_i |x|, 1e-8, inf)                 (per token)
  x_q     = clip(round(x * 127 / a), -127, 127)        (8-bit ints)
  y       = (x_q @ w_q^T) * w_scale * a / 127

All rounding uses the fp32 magic-number trick (v + 1.5*2^23 - 1.5*2^23 is
round-to-nearest-even). x_q (ints <= 127) is exact in bf16; w_q ({-1,0,1})
is exact in fp8e4; their products accumulate exactly in fp32 PSUM, so the
TensorE matmul (bf16 stationary x fp8 moving — verified exact on HW) is
exact. y is stored bf16 (~3e-3 max-normalized rounding, budget is 2e-2).

Schedule (single pass over W, GEMM starts ~50us in):
  pass 1   load all 16 W row-tiles once, |.|-row-sums on DVE. The last 4
           tiles stay resident in SBUF; the rest stream through a small pool.
  scale    partition-reduce (GpSimd) + scalar chain -> w_scale, 1/w_scale.
  pass 2a  quantize the 4 resident tiles (quarter no=3) with zero DMA:
           scalar act (round magic) -> DVE clip -> scalar act (-CM, bf16)
           -> bf16 DMA transpose straight into wqT. GEMM can start here.
           (GpSimd bulk tensor ops are ~9 G elem/s AND stall concurrent DVE
           ops to ~28us — measured; gpsimd only issues W-transpose DMAs.)
  pass 2b  re-read j8..11, 4..7, 0..3 (quarters 2,1,0) interleaved with the
           x-tile preps; deadlines are one GEMM sweep apart so the re-reads
           ride spare DMA bandwidth behind the x loads.
  main     for no in 3,2,1,0: for t in 0..15: 16 matmuls -> PSUM, DVE
           drain *(a*w_scale/127) -> bf16 y store. wqT quarters rotate
           through a 2-buffer pool (no=1 overwrites no=3 after its sweep).
"""

from contextlib import ExitStack

import numpy as np

import concourse.bass as bass
import concourse.tile as tile
from concourse import bacc, bass_isa, mybir
from concourse.bass import ds, ts
from concourse.bass_utils import run_bass_kernel_spmd

F32 = mybir.dt.float32
BF16 = mybir.dt.bfloat16
FP8 = mybir.dt.float8e4
AF = mybir.ActivationFunctionType
OP = mybir.AluOpType
AX = mybir.AxisListType

B, S, D_IN, D_OUT = 4, 4096, 2048, 2048
N_CORES = 8
TOK = B * S                # 16384 tokens
TPC = TOK // N_CORES       # 2048 tokens per core
NT = TPC // 128            # 16 token tiles per core
NJ = D_OUT // 128          # 16 weight row tiles
NI = D_IN // 128           # 16 contraction (k) blocks
NO = D_OUT // 512          # 4 output column blocks
CM = 12582912.0            # 1.5 * 2^23: fp32 RNE rounding magic
QMAX = 127.0

KNOBS = {
    "ld_bufs": 3,          # streamed W tiles
    "ldx_bufs": 2,         # streamed x tiles
    "wq_bufs": 4,          # quantized-W bf16 staging (parks tiles while the
                           # next quarter's transposes wait on sweep reads)
    "xq_bufs": 2,          # quantized-x bf16 staging
    "wqt_bufs": 2,         # rotating wqT quarter buffers
    "ys_bufs": 3,
    "n_res": 4,            # W tiles kept resident from pass 1 (quarter no=3)
    "pref_pass1": 2,       # x preps emitted inside pass 1
}

_CACHE = {}


def _emit(tc: tile.TileContext, x_d: bass.AP, w_d: bass.AP, y_d: bass.AP):
    nc = tc.nc
    K = KNOBS
    with ExitStack() as ctx:
        ld = ctx.enter_context(tc.tile_pool(name="ld", bufs=K["ld_bufs"]))
        ldx = ctx.enter_context(tc.tile_pool(name="ldx", bufs=K["ldx_bufs"]))
        wres = ctx.enter_context(tc.tile_pool(name="wres", bufs=1))
        wqp = ctx.enter_context(tc.tile_pool(name="wqp", bufs=K["wq_bufs"]))
        xqp = ctx.enter_context(tc.tile_pool(name="xqp", bufs=K["xq_bufs"]))
        wqtp = ctx.enter_context(tc.tile_pool(name="wqtp", bufs=K["wqt_bufs"]))
        xqtp = ctx.enter_context(tc.tile_pool(name="xqtp", bufs=1))
        ysp = ctx.enter_context(tc.tile_pool(name="ysp", bufs=K["ys_bufs"]))
        stats = ctx.enter_context(tc.tile_pool(name="stats", bufs=6))
        consts = ctx.enter_context(tc.tile_pool(name="consts", bufs=1))
        wsp = ctx.enter_context(tc.tile_pool(name="wsp", bufs=1))
        psum = ctx.enter_context(
            tc.tile_pool(name="psum", bufs=8, space=bass.MemorySpace.PSUM)
        )

        cpos = consts.tile([128, 1], F32, tag="cpos")
        nc.vector.memset(cpos, CM)
        cneg = consts.tile([128, 1], F32, tag="cneg")
        nc.vector.memset(cneg, -CM)

        NRES = K["n_res"]
        jres0 = NJ - NRES                       # first resident j (12)

        # persistent per-token-tile tiles
        xqT = [
            xqtp.tile([128, NI, 128], BF16, tag=f"xqT{t}", name=f"xqT{t}")
            for t in range(NT)
        ]
        souts = [
            consts.tile([128, 1], F32, tag=f"sout{t}", name=f"sout{t}")
            for t in range(NT)
        ]
        # transposed quantized W quarters [128, 4(jq), NI, 128] bf16, lazily
        # allocated from a rotating pool in quantization order 3,2,1,0
        wqT = {}

        ws127 = consts.tile([128, 1], F32, tag="ws127")
        rws = consts.tile([128, 1], F32, tag="rws")

        a_tiles = {}

        def x_prep(t, ws_ready=True):
            """load x tile t, compute scales, quantize, transpose to xqT[t]."""
            xt = ldx.tile([128, D_IN], F32, tag="ldx", name=f"xt{t}")
            nc.sync.dma_start(xt, x_d[ts(t, 128), :])
            a = consts.tile([128, 1], F32, tag=f"xa{t}", name=f"xa{t}")
            a_tiles[t] = a
            nc.vector.reduce_max(a, xt, axis=AX.X, apply_absolute_value=True)
            nc.vector.tensor_scalar(a, a, 1e-8, None, OP.max)
            r0 = stats.tile([128, 1], F32, tag="xr0", name=f"xr0{t}")
            nc.vector.reciprocal(r0, a)
            ntt = stats.tile([128, 1], F32, tag="xntt", name=f"xntt{t}")
            nc.vector.tensor_mul(ntt, a, r0)
            nc.vector.tensor_scalar(ntt, ntt, -1.0, 2.0, OP.mult, OP.add)
            s = stats.tile([128, 1], F32, tag="xs", name=f"xs{t}")
            nc.vector.tensor_mul(s, r0, ntt)
            nc.vector.tensor_scalar(s, s, QMAX, None, OP.mult)  # 127/a
            if ws_ready:
                nc.vector.tensor_mul(souts[t], a, ws127)  # a * w_scale / 127

            # t1 = x*(127/a) + CM  (in-place; RNE round via fp32 magic add)
            nc.scalar.activation(xt, xt, AF.Identity, bias=cpos, scale=s)
            xq = xqp.tile([128, D_IN], BF16, tag="xq", name=f"xq{t}")
            nc.vector.tensor_scalar(xq, xt, -CM, None, OP.add)
            nc.scalar.dma_start(xqT[t], xq, transpose=True)

        def w_quant(j, wt):
            """quantize W row-tile j (already in SBUF as wt) into wqT."""
            no, jq = j // 4, j % 4
            if no not in wqT:
                wqT[no] = wqtp.tile(
                    [128, NJ // NO, NI, 128], BF16, tag="wqT", name=f"wqT{no}"
                )
            # t1 = W/ws + CM (in-place), clip in offset domain, -CM -> bf16
            nc.scalar.activation(wt, wt, AF.Identity, bias=cpos, scale=rws)
            nc.vector.tensor_scalar(wt, wt, CM - 1.0, CM + 1.0, OP.max, OP.min)
            wq = wqp.tile([128, D_IN], BF16, tag="wq", name=f"wq{j}")
            nc.scalar.activation(wq, wt, AF.Identity, bias=cneg)
            nc.scalar.dma_start(wqT[no][:, jq, :, :], wq, transpose=True)

        # ---- pass 1: single read of W, |.| row-sums on DVE ----
        wsums = wsp.tile([128, NJ], F32, tag="wsums")
        wres_tiles = {}
        for j in range(NJ):
            if j >= jres0:
                wt = wres.tile([128, D_IN], F32, tag=f"wres{j}", name=f"wres{j}")
                wres_tiles[j] = wt
            else:
                wt = ld.tile([128, D_IN], F32, tag="ld", name=f"wp1_{j}")
            nc.sync.dma_start(wt, w_d[ts(j, 128), :])
            if j >= jres0:
                # non-destructive: this tile is quantized from SBUF later
                nc.vector.reduce_sum(
                    wsums[:, ds(j, 1)], wt, axis=AX.X, apply_absolute_value=True
                )
            else:
                czero = consts.tile([128, 1], F32, tag=f"cz{j}", name=f"cz{j}")
                nc.vector.memset(czero, 0.0)
                nc.scalar.activation(
                    wt, wt, AF.Abs, bias=czero, accum_out=wsums[:, ds(j, 1)]
                )
            if j == 7 and K["pref_pass1"] >= 1:
                x_prep(0, ws_ready=False)
            if j == 11 and K["pref_pass1"] >= 2:
                x_prep(1, ws_ready=False)

        # ---- w_scale = max(mean|W|, 1e-6), rws ~ 1/w_scale (Newton) ----
        wsum_p = stats.tile([128, 1], F32, tag="wsp")
        nc.vector.reduce_sum(wsum_p, wsums, axis=AX.X)
        wsum_all = stats.tile([128, 1], F32, tag="wsa")
        nc.gpsimd.partition_all_reduce(wsum_all, wsum_p, 128, bass_isa.ReduceOp.add)
        wscale = stats.tile([128, 1], F32, tag="wscale")
        nc.vector.tensor_scalar(
            wscale, wsum_all, 1.0 / (D_OUT * D_IN), 1e-6, OP.mult, OP.max
        )
        r0 = stats.tile([128, 1], F32, tag="wr0")
        nc.vector.reciprocal(r0, wscale)
        ntt = stats.tile([128, 1], F32, tag="wntt")
        nc.vector.tensor_mul(ntt, wscale, r0)
        nc.vector.tensor_scalar(ntt, ntt, -1.0, 2.0, OP.mult, OP.add)
        nc.vector.tensor_mul(rws, r0, ntt)
        nc.vector.tensor_scalar(ws127, wscale, 1.0 / QMAX, None, OP.mult)
        for t in range(min(K["pref_pass1"], NT)):
            nc.vector.tensor_mul(souts[t], a_tiles[t], ws127)

        # ---- pass 2a: resident quarter (no=3), zero DMA on critical path ----
        for j in range(jres0, NJ):
            w_quant(j, wres_tiles[j])

        x_prep(2)
        x_prep(3)

        # ---- pass 2b: re-read the rest in sweep order 2, 1, 0;
        #      interleave remaining x preps (x loads get ring priority) ----
        def w_requant(j):
            wt = ld.tile([128, D_IN], F32, tag="ld", name=f"wp2_{j}")
            nc.sync.dma_start(wt, w_d[ts(j, 128), :])
            w_quant(j, wt)

        xi = 4
        for jg in (range(8, 12), range(4, 8), range(0, 4)):
            for j in jg:
                if xi < NT:
                    x_prep(xi)
                    xi += 1
                w_requant(j)
        while xi < NT:
            x_prep(xi)
            xi += 1

        # ---- main GEMM: no-major sweeps, newest quarter first ----
        for no in range(NO - 1, -1, -1):
            for t in range(NT):
                ps = psum.tile([128, 512], F32, tag="ps")
                for b in range(NI):
                    nc.tensor.matmul(
                        ps,
                        xqT[t][:, b, :],
                        wqT[no][:, :, b, :],
                        start=(b == 0),
                        stop=(b == NI - 1),
                    )
                ys = ysp.tile([128, 512], BF16, tag="ys")
                nc.vector.tensor_scalar(ys, ps, souts[t], None, OP.mult)
                nc.sync.dma_start(y_d[ts(t, 128), ds(no * 512, 512)], ys)


def _build():
    key = tuple(sorted((k, str(v)) for k, v in KNOBS.items()))
    if key in _CACHE:
        return _CACHE[key]
    nc = bacc.Bacc(
        "TRN2", target_bir_lowering=False, debug=False, num_devices=N_CORES
    )
    x_d = nc.dram_tensor("x", [TPC, D_IN], F32, kind="ExternalInput").ap()
    w_d = nc.dram_tensor("w", [D_OUT, D_IN], F32, kind="ExternalInput").ap()
    y_d = nc.dram_tensor("y", [TPC, D_OUT], BF16, kind="ExternalOutput").ap()
    with tile.TileContext(nc) as tc:
        _emit(tc, x_d, w_d, y_d)
    nc.compile()
    _CACHE[key] = nc
    return nc


_last_result = None  # BassKernelResults of the most recent run (for profiling)


def kernel(x: np.ndarray, weight: np.ndarray, trace: bool = False) -> np.ndarray:
    global _last_result
    nc = _build()
    xf = np.ascontiguousarray(x.reshape(TOK, D_IN), dtype=np.float32)
    wf = np.ascontiguousarray(weight, dtype=np.float32)
    in_maps = [
        {"x": xf[c * TPC:(c + 1) * TPC], "w": wf} for c in range(N_CORES)
    ]
    res = run_bass_kernel_spmd(nc, in_maps, list(range(N_CORES)), trace=trace)
    _last_result = res
    y = np.concatenate(
        [np.asarray(res.results[c]["y"]) for c in range(N_CORES)], axis=0
    )
    return y.astype(np.float32).reshape(B, S, D_OUT)
